# revision 1
# baseline (speedup 1.0000x reference)
"""Trainium2 Bass kernel for nn_MBRNNIncrementEstimator (GRU increment estimator).

Model (per batch b):
  X_prior[t] = F^{t+1} x0                       (linear prior scan)
  x_proj     = concat(Y, X_prior) @ W_ih.T + b_ih
  GRU over T with W_hh, b_hh  -> outs
  out        = X_prior + outs @ fc_W.T + fc_b

Sharding: data-parallel over batch B=64 across 8 cores (8 batches/core).
All on-chip compute uses a transposed layout (features on partitions) so
the GRU per-step vector math uses all 128 lanes.

Key design points:
 - The input-projection GEMM writes gate pre-activations directly into the
   PSUM banks the recurrent matmuls later accumulate into (start=False), so
   the GRU inner loop needs no DMA and no separate adds.
 - Weights are bf16 (FWL 2x weight load); accumulation is fp32 in PSUM.
 - Biases are folded in with K=1 rank-1 matmuls (bias x ones).
 - The prior scan is blocked using host-precomputed powers of F (weight-only
   preprocessing), turning 1024 sequential steps into ~80 small matmuls.
"""

import os
import numpy as np
import ml_dtypes

B, T, NOBS, MST, HID = 64, 1024, 64, 64, 512
H3 = 3 * HID
NCORES = 8
BS = B // NCORES            # 8 batches per core
CB = 64                     # prior-scan block length
JP = T // CB                # 16 prior blocks
C = 8                       # GRU psum block length
NBLK = T // C

_compiled = {}
LAST_RESULTS = None


def _build_bass(t_steps):
    import concourse.bass as bass
    import concourse.mybir as mybir
    import concourse.tile as tile
    from concourse import bacc
    from concourse.masks import make_identity

    f32 = mybir.dt.float32
    bf16 = mybir.dt.bfloat16
    wdt = mybir.dt.float8e4 if os.environ.get("KW8") == "1" else bf16

    Tt = t_steps
    nblk = Tt // C
    jp = max(1, Tt // CB)

    nc = bacc.Bacc(None, target_bir_lowering=False)
    Y_d = nc.declare_dram_parameter("Y", [BS, Tt, NOBS], f32, isOutput=False)
    x0T_d = nc.declare_dram_parameter("x0T", [MST, BS], f32, isOutput=False)
    FkT_d = nc.declare_dram_parameter("FkT", [MST, 65 * MST], f32, isOutput=False)
    WihT_d = nc.declare_dram_parameter("WihT", [128, H3], bf16, isOutput=False)
    WhhT_d = nc.declare_dram_parameter("WhhT", [128, 48 * 128], wdt, isOutput=False)
    bMv_d = nc.declare_dram_parameter("bMv", [128, 12], f32, isOutput=False)
    bhnT_d = nc.declare_dram_parameter("bhnT", [128, 32], f32, isOutput=False)
    fcWT_d = nc.declare_dram_parameter("fcWT", [128, 4 * MST], bf16, isOutput=False)
    fcb_d = nc.declare_dram_parameter("fcb", [MST, 1], f32, isOutput=False)
    out_d = nc.declare_dram_parameter("out", [BS, Tt, MST], f32, isOutput=True)
    dbg = os.environ.get("KDBG") == "1"
    if dbg:
        dbg_d = nc.declare_dram_parameter("dbg", [128, 768], f32, isOutput=True)

    NT = Tt * BS  # columns in transposed (b-major) layout

    with tile.TileContext(nc) as tc:
        with (
            tc.tile_pool(name="singles", bufs=1) as singles,
            tc.tile_pool(name="yload", bufs=4) as yload,
            tc.tile_pool(name="work", bufs=4) as work,
            tc.tile_pool(name="xps", bufs=2) as xps,
            tc.tile_pool(name="pprior", bufs=2, space="PSUM") as pprior,
            tc.tile_pool(name="pgates", bufs=1, space="PSUM") as pgates,
            tc.tile_pool(name="pfc", bufs=1, space="PSUM") as pfc,
        ):
            # ---- resident tensors ----
            wih = singles.tile([128, H3], bf16)
            whh = singles.tile([128, 48 * 128], wdt)
            fcw = singles.tile([128, 4 * MST], bf16)
            bmv = singles.tile([128, 12], f32)
            bhnt = singles.tile([128, 32], f32)
            fcb = singles.tile([MST, 1], f32)
            fkt = singles.tile([MST, 65 * MST], f32)
            x0t = singles.tile([MST, BS], f32)
            ident = singles.tile([128, 128], f32)
            inpT = singles.tile([128, NT], bf16)
            XpT = singles.tile([MST, NT], f32)
            outsT = singles.tile([128, (Tt + 1) * 32], bf16)
            S_all = singles.tile([MST, 128], f32)

            nc.sync.dma_start(wih[:], WihT_d[:])
            nc.sync.dma_start(whh[:], WhhT_d[:])
            nc.sync.dma_start(fcw[:], fcWT_d[:])
            nc.sync.dma_start(bmv[:], bMv_d[:])
            nc.sync.dma_start(bhnt[:], bhnT_d[:])
            nc.sync.dma_start(fcb[:], fcb_d[:])
            nc.sync.dma_start(fkt[:], FkT_d[:])
            nc.sync.dma_start(x0t[:], x0T_d[:])
            make_identity(nc, ident[:])
            nc.vector.memset(outsT[:, 0:32], 0.0)  # h_0 = 0

            def fk(k):  # (F^k)^T as [64,64] lhsT slice
                return fkt[:, k * MST:(k + 1) * MST]

            # ---- phase 1: prior scan (blocked) ----
            # S_all[:, j*8:+8] = X_prior[:, j*CB]^T ; S_0 = F x0
            ps = pprior.tile([MST, BS], f32, tag="pp")
            nc.tensor.matmul(ps[:], fk(1), x0t[:], start=True, stop=True)
            nc.vector.tensor_copy(S_all[:, 0:BS], ps[:])
            for j in range(1, jp):
                ps = pprior.tile([MST, BS], f32, tag="pp")
                nc.tensor.matmul(
                    ps[:], fk(CB), S_all[:, (j - 1) * BS:j * BS], start=True, stop=True
                )
                nc.vector.tensor_copy(S_all[:, j * BS:(j + 1) * BS], ps[:])

            # Xp cols for t = j*CB + k  (all j at once per k)
            def xp_dst(a, k):
                # dst AP over cols {b*Tt + j*CB + k}: (j outer, b inner)
                return bass.AP(
                    tensor=a.tensor,
                    offset=a.offset + k,
                    ap=[list(a.ap[0]), [CB, jp], [Tt, BS]],
                )

            for k in range(CB):
                if Tt < CB and k >= Tt:
                    break
                if k == 0:
                    src = S_all[:, 0:jp * BS]
                else:
                    psk = pprior.tile([MST, jp * BS], f32, tag="pp")
                    nc.tensor.matmul(
                        psk[:], fk(k), S_all[:, 0:jp * BS], start=True, stop=True
                    )
                    src = psk[:]
                src3 = bass.AP(
                    tensor=src.tensor, offset=src.offset,
                    ap=[list(src.ap[0]), [BS, jp], [1, BS]],
                )
                nc.vector.tensor_copy(xp_dst(XpT[:], k), src3)
                nc.vector.tensor_copy(xp_dst(inpT[64:128, :], k), src3)

            # ---- phase 2: Y^T into inpT rows 0:64 ----
            for b in range(BS):
                for tch in range(Tt // 128):
                    yt = yload.tile([128, NOBS], f32, tag="yt")
                    nc.sync.dma_start(yt[:], Y_d[b, tch * 128:(tch + 1) * 128, :])
                    pyt = pprior.tile([NOBS, 128], f32, tag="pp")
                    nc.tensor.transpose(pyt[:], yt[:], ident[:])
                    nc.vector.tensor_copy(
                        inpT[0:64, b * Tt + tch * 128: b * Tt + (tch + 1) * 128],
                        pyt[:],
                    )

            # ---- phase 3: GRU ----
            # Separate psum tensors, homogeneous accumulation groups only:
            #   xp_ps [128, 12*R]: x-projection GEMM output (one MM per region)
            #   rz_ps [128, 8*R]:  W_hh r/z recurrent accumulation (kc groups)
            #   hn_ps [128, 4*R]:  W_hh n recurrent accumulation
            # xp then moves to SBUF with per-region bias via tensor_scalar_add.
            R = C * 8
            xp_ps = pgates.tile([128, 12 * R], f32, tag="xp_ps")
            rz_ps = pgates.tile([128, 8 * R], f32, tag="rz_ps")
            hn_ps = pgates.tile([128, 4 * R], f32, tag="hn_ps")

            def kslice(ap_full, k, nreg):
                return bass.AP(
                    tensor=ap_full.tensor,
                    offset=ap_full.offset + k * 8,
                    ap=[list(ap_full.ap[0]), [R, nreg], [1, 8]],
                )

            def hslot(t):
                return outsT[:, t * 32:(t + 1) * 32]

            kreps = int(os.environ.get("KREPS", "1"))
            for rep in range(kreps):
              for j in range(nblk):
                t0 = j * C
                ia = inpT[:]
                rhs_inp = bass.AP(
                    tensor=ia.tensor, offset=ia.offset + t0,
                    ap=[list(ia.ap[0]), [1, C], [Tt, BS]],
                )
                xp_sb = xps.tile([128, 12 * R], f32, tag="xp")
                for m in range(12):
                    nc.tensor.matmul(
                        xp_ps[:, m * R:(m + 1) * R],
                        wih[:, m * 128:(m + 1) * 128],
                        rhs_inp,
                        start=True, stop=True,
                    )
                    nc.vector.tensor_scalar_add(
                        xp_sb[:, m * R:(m + 1) * R],
                        xp_ps[:, m * R:(m + 1) * R],
                        scalar1=bmv[:, m:m + 1],
                    )

                for k in range(C):
                    t = t0 + k
                    h_rd = hslot(t)

                    for i in range(4):      # hn first (n-path is latency-critical)
                        for kc in range(4):
                            nc.tensor.matmul(
                                hn_ps[:, i * R + k * 8:i * R + k * 8 + 8],
                                whh[:, (kc * 12 + 8 + i) * 128:(kc * 12 + 9 + i) * 128],
                                h_rd[:, kc * 8:(kc + 1) * 8],
                                start=(kc == 0), stop=(kc == 3),
                            )
                    for mi in range(8):     # r then z tiles
                        for kc in range(4):
                            nc.tensor.matmul(
                                rz_ps[:, mi * R + k * 8:mi * R + k * 8 + 8],
                                whh[:, (kc * 12 + mi) * 128:(kc * 12 + mi + 1) * 128],
                                h_rd[:, kc * 8:(kc + 1) * 8],
                                start=(kc == 0), stop=(kc == 3),
                            )

                    rzs = work.tile([128, 64], f32, tag="rzs")
                    nc.vector.tensor_add(rzs[:], kslice(rz_ps[:], k, 8), kslice(xp_sb[:], k, 8))
                    rza = work.tile([128, 64], f32, tag="rza")
                    nc.scalar.activation(rza[:], rzs[:], mybir.ActivationFunctionType.Sigmoid)
                    th = work.tile([128, 32], f32, tag="th")
                    nc.vector.tensor_add(th[:], kslice(hn_ps[:], k, 4), bhnt[:])
                    t1 = work.tile([128, 32], f32, tag="t1")
                    nc.vector.tensor_mul(t1[:], rza[:, 0:32], th[:])
                    t2 = work.tile([128, 32], f32, tag="t2")
                    xn_sl = bass.AP(
                        tensor=xp_sb[:].tensor,
                        offset=xp_sb[:].offset + 8 * R + k * 8,
                        ap=[list(xp_sb[:].ap[0]), [R, 4], [1, 8]],
                    )
                    nc.vector.tensor_add(t2[:], t1[:], xn_sl)
                    n_t = work.tile([128, 32], f32, tag="n")
                    nc.scalar.activation(n_t[:], t2[:], mybir.ActivationFunctionType.Tanh)
                    d_t = work.tile([128, 32], f32, tag="d")
                    nc.vector.tensor_sub(d_t[:], h_rd, n_t[:])
                    zd = work.tile([128, 32], f32, tag="zd")
                    nc.vector.tensor_mul(zd[:], rza[:, 32:64], d_t[:])
                    nc.vector.tensor_add(hslot(t + 1), n_t[:], zd[:])

            # ---- phase 4: fc + X_prior + output ----
            for b in range(BS):
                for half in range(max(1, Tt // 512)):
                    tw = min(512, Tt)
                    t0 = half * 512
                    psfc = pfc.tile([MST, tw], f32, tag="fc")
                    for kc in range(4):
                        oa = outsT[:]
                        rhs = bass.AP(
                            tensor=oa.tensor,
                            offset=oa.offset + (t0 + 1) * 32 + kc * 8 + b,
                            ap=[list(oa.ap[0]), [32, tw]],
                        )
                        # b index: outsT free = (t, kcgroup m, b): col = t*32+m*8+b
                        nc.tensor.matmul(
                            psfc[:], fcw[:, kc * MST:(kc + 1) * MST], rhs,
                            start=(kc == 0), stop=(kc == 3),
                        )
                    oT = work.tile([MST, tw], f32, tag="oT")
                    nc.vector.scalar_tensor_tensor(
                        oT[:], psfc[:], fcb[:], XpT[:, b * Tt + t0: b * Tt + t0 + tw],
                        op0=mybir.AluOpType.add, op1=mybir.AluOpType.add,
                    )
                    for q in range(tw // 128):
                        ptr = pfc.tile([128, MST], f32, tag="tr")
                        nc.tensor.transpose(
                            ptr[:], oT[:, q * 128:(q + 1) * 128], ident[0:64, 0:64]
                        )
                        ot = work.tile([128, MST], f32, tag="ot")
                        nc.vector.tensor_copy(ot[:], ptr[:])
                        nc.sync.dma_start(
                            out_d[b, t0 + q * 128: t0 + (q + 1) * 128, :], ot[:]
                        )

    nc.compile()
    return nc


def _prep_weights(F_mat, W_ih, W_hh, b_ih, b_hh, fc_W, fc_b):
    bf = ml_dtypes.bfloat16
    FkT = np.empty((MST, 65 * MST), np.float32)
    P = np.eye(MST, dtype=np.float32)
    for k in range(65):
        FkT[:, k * MST:(k + 1) * MST] = P.T
        P = (F_mat @ P).astype(np.float32)
    WihT = np.ascontiguousarray(W_ih.T).astype(bf)
    wnp = ml_dtypes.float8_e4m3 if os.environ.get("KW8") == "1" else bf
    WhhT = np.empty((128, 48 * 128), wnp)
    for kc in range(4):
        for m in range(12):
            blk = W_hh[m * 128:(m + 1) * 128, kc * 128:(kc + 1) * 128]
            WhhT[:, (kc * 12 + m) * 128:(kc * 12 + m + 1) * 128] = blk.T.astype(wnp)
    bM = (b_ih + np.concatenate([b_hh[:2 * HID], np.zeros(HID, np.float32)]))
    bMv = bM.reshape(12, 128).T.astype(np.float32).copy()          # [128, 12]
    bN = b_hh[2 * HID:].reshape(4, 128).T.astype(np.float32)       # [128, 4]
    bhnT = np.repeat(bN, 8, axis=1).astype(np.float32)             # [128, (i,b)=32]
    fcWT = np.empty((128, 4 * MST), bf)
    for kc in range(4):
        fcWT[:, kc * MST:(kc + 1) * MST] = fc_W[:, kc * 128:(kc + 1) * 128].T.astype(bf)
    fcb = fc_b.reshape(MST, 1).astype(np.float32)
    return dict(FkT=FkT, WihT=WihT, WhhT=WhhT, bMv=bMv, bhnT=bhnT,
                fcWT=fcWT, fcb=fcb)


def kernel(Y, x0_hat, F_mat, W_ih, W_hh, b_ih, b_hh, fc_W, fc_b):
    from concourse.bass_utils import run_bass_kernel_spmd

    t_steps = Y.shape[1]
    if t_steps not in _compiled:
        _compiled[t_steps] = _build_bass(t_steps)
    nc = _compiled[t_steps]

    w = _prep_weights(F_mat, W_ih, W_hh, b_ih, b_hh, fc_W, fc_b)
    in_maps = []
    for c in range(NCORES):
        sl = slice(c * BS, (c + 1) * BS)
        in_maps.append({
            "Y": np.ascontiguousarray(Y[sl]).astype(np.float32),
            "x0T": np.ascontiguousarray(x0_hat[sl].T).astype(np.float32),
            **w,
        })
    trace = os.environ.get("KTRACE") == "1"
    res = run_bass_kernel_spmd(nc, in_maps, list(range(NCORES)), trace=trace)
    global LAST_RESULTS
    LAST_RESULTS = res
    out = np.concatenate([res.results[c]["out"] for c in range(NCORES)], axis=0)
    return out.astype(np.float32)


if __name__ == "__main__":
    rng = np.random.default_rng(0)
    ins = {
        "Y": rng.standard_normal((B, int(os.environ.get("KT", T)), NOBS), dtype=np.float32),
        "x0_hat": rng.standard_normal((B, MST), dtype=np.float32),
        "F_mat": (0.99 * np.linalg.qr(rng.standard_normal((MST, MST)))[0]).astype(np.float32),
        "W_ih": 0.05 * rng.standard_normal((H3, 128), dtype=np.float32),
        "W_hh": 0.05 * rng.standard_normal((H3, HID), dtype=np.float32),
        "b_ih": 0.05 * rng.standard_normal(H3, dtype=np.float32),
        "b_hh": 0.05 * rng.standard_normal(H3, dtype=np.float32),
        "fc_W": 0.05 * rng.standard_normal((MST, HID), dtype=np.float32),
        "fc_b": 0.05 * rng.standard_normal(MST, dtype=np.float32),
    }
    print(kernel(**ins).shape)



# revision 4
# speedup vs baseline: 1.2440x; 1.2440x over previous
"""Trainium2 Bass kernel for nn_MBRNNIncrementEstimator (GRU increment estimator).

Model (per batch b):
  X_prior[t] = F^{t+1} x0                       (linear prior scan)
  x_proj     = concat(Y, X_prior) @ W_ih.T + b_ih
  GRU over T with W_hh, b_hh  -> outs
  out        = X_prior + outs @ fc_W.T + fc_b

Sharding: data-parallel over batch B=64 across 8 cores (8 batches/core).
All on-chip compute uses a transposed layout (features on partitions) so
the GRU per-step vector math uses all 128 lanes.

v2 design (vs the first working version):
 - Gate pre-activations live entirely in PSUM: each block's PSUM bank is
   initialized by ONE K=8 "indicator" matmul that broadcasts the per-tile
   biases bank-wide (start=True zeroes the 2KB zero region), then the
   block-level input-projection GEMM and the per-step recurrent matmuls
   accumulate on top (start=False).  This removes three DVE ops from the
   per-step serial chain (x-proj add, hn bias add, x-proj bias/copy).
 - Per-step chain is 5 ops: sigmoid(rz PSUM) -> t1 = r*hn -> t2 = t1+xn
   -> n = tanh(t2) -> h' = n*(1-z) + z*h, with (1-z) computed on the
   Scalar engine and z*h on Vector while tanh runs.
 - Burst order r,z then hn so the sigmoid overlaps the hn matmuls.
 - fc output GEMM + PE transpose + DMA are emitted per-block so they fill
   the tensor-engine idle window during each step's vector tail.
"""

import os
import numpy as np
import ml_dtypes

B, T, NOBS, MST, HID = 64, 1024, 64, 64, 512
H3 = 3 * HID
NCORES = 8
BS = B // NCORES            # 8 batches per core
CB = 64                     # prior-scan block length
C = 8                       # GRU block length (steps per PSUM block)

_compiled = {}
LAST_RESULTS = None


def _build_bass(t_steps):
    import concourse.bass as bass
    import concourse.mybir as mybir
    import concourse.tile as tile
    from concourse import bacc
    from concourse.masks import make_identity

    f32 = mybir.dt.float32
    bf16 = mybir.dt.bfloat16

    Tt = t_steps
    nblk = Tt // C
    jp = max(1, Tt // CB)

    nc = bacc.Bacc(None, target_bir_lowering=False)
    Y_d = nc.declare_dram_parameter("Y", [BS, Tt, NOBS], f32, isOutput=False)
    x0T_d = nc.declare_dram_parameter("x0T", [MST, BS], f32, isOutput=False)
    FkT_d = nc.declare_dram_parameter("FkT", [MST, 65 * MST], f32, isOutput=False)
    WihT_d = nc.declare_dram_parameter("WihT", [128, H3], bf16, isOutput=False)
    WhhT_d = nc.declare_dram_parameter("WhhT", [128, 48 * 128], bf16, isOutput=False)
    brz8_d = nc.declare_dram_parameter("brz8", [8, 128], bf16, isOutput=False)
    bnx8_d = nc.declare_dram_parameter("bnx8", [8, 128], bf16, isOutput=False)
    ind_d = nc.declare_dram_parameter("ind", [8, 512], bf16, isOutput=False)
    fcWT_d = nc.declare_dram_parameter("fcWT", [128, 4 * MST], bf16, isOutput=False)
    fcb_d = nc.declare_dram_parameter("fcb", [MST, 1], f32, isOutput=False)
    out_d = nc.declare_dram_parameter("out", [BS, Tt, MST], f32, isOutput=True)

    NT = Tt * BS  # columns in transposed (b-major) layout

    with tile.TileContext(nc) as tc:
        with (
            tc.tile_pool(name="singles", bufs=1) as singles,
            tc.tile_pool(name="yload", bufs=4) as yload,
            tc.tile_pool(name="work", bufs=4) as work,
            tc.tile_pool(name="pprior", bufs=2, space="PSUM") as pprior,
            tc.tile_pool(name="pgates", bufs=2, space="PSUM") as pgates,
            tc.tile_pool(name="pfc", bufs=1, space="PSUM") as pfc,
        ):
            # ---- resident tensors ----
            wih = singles.tile([128, H3], bf16)
            whh = singles.tile([128, 48 * 128], bf16)
            fcw = singles.tile([128, 4 * MST], bf16)
            brz8 = singles.tile([8, 128], bf16)
            bnx8 = singles.tile([8, 128], bf16)
            ind = singles.tile([8, 512], bf16)
            fcb = singles.tile([MST, 1], f32)
            fkt = singles.tile([MST, 65 * MST], f32)
            x0t = singles.tile([MST, BS], f32)
            ident = singles.tile([128, 128], f32)
            inpT = singles.tile([128, NT], bf16)
            XpT = singles.tile([MST, NT], f32)
            outsT = singles.tile([128, (Tt + 1) * 32], bf16)
            S_all = singles.tile([MST, 128], f32)

            nc.sync.dma_start(wih[:], WihT_d[:])
            nc.sync.dma_start(whh[:], WhhT_d[:])
            nc.sync.dma_start(fcw[:], fcWT_d[:])
            nc.sync.dma_start(brz8[:], brz8_d[:])
            nc.sync.dma_start(bnx8[:], bnx8_d[:])
            nc.sync.dma_start(ind[:], ind_d[:])
            nc.sync.dma_start(fcb[:], fcb_d[:])
            nc.sync.dma_start(fkt[:], FkT_d[:])
            nc.sync.dma_start(x0t[:], x0T_d[:])
            make_identity(nc, ident[:])
            nc.vector.memset(outsT[:, 0:32], 0.0)  # h_0 = 0

            def fk(k):  # (F^k)^T as [64,64] lhsT slice
                return fkt[:, k * MST:(k + 1) * MST]

            # ---- phase 1: prior scan (blocked) ----
            # S_all[:, j*8:+8] = X_prior[:, j*CB]^T ; S_0 = F x0
            ps = pprior.tile([MST, BS], f32, tag="pp")
            nc.tensor.matmul(ps[:], fk(1), x0t[:], start=True, stop=True)
            nc.vector.tensor_copy(S_all[:, 0:BS], ps[:])
            for j in range(1, jp):
                ps = pprior.tile([MST, BS], f32, tag="pp")
                nc.tensor.matmul(
                    ps[:], fk(CB), S_all[:, (j - 1) * BS:j * BS], start=True, stop=True
                )
                nc.vector.tensor_copy(S_all[:, j * BS:(j + 1) * BS], ps[:])

            # Xp cols for t = j*CB + k  (all j at once per k)
            def xp_dst(a, k):
                # dst AP over cols {b*Tt + j*CB + k}: (j outer, b inner)
                return bass.AP(
                    tensor=a.tensor,
                    offset=a.offset + k,
                    ap=[list(a.ap[0]), [CB, jp], [Tt, BS]],
                )

            for k in range(CB):
                if Tt < CB and k >= Tt:
                    break
                if k == 0:
                    src = S_all[:, 0:jp * BS]
                else:
                    psk = pprior.tile([MST, jp * BS], f32, tag="pp")
                    nc.tensor.matmul(
                        psk[:], fk(k), S_all[:, 0:jp * BS], start=True, stop=True
                    )
                    src = psk[:]
                src3 = bass.AP(
                    tensor=src.tensor, offset=src.offset,
                    ap=[list(src.ap[0]), [BS, jp], [1, BS]],
                )
                nc.vector.tensor_copy(xp_dst(XpT[:], k), src3)
                nc.vector.tensor_copy(xp_dst(inpT[64:128, :], k), src3)

            # ---- phase 2: Y^T into inpT rows 0:64 ----
            for b in range(BS):
                for tch in range(Tt // 128):
                    yt = yload.tile([128, NOBS], f32, tag="yt")
                    nc.sync.dma_start(yt[:], Y_d[b, tch * 128:(tch + 1) * 128, :])
                    pyt = pprior.tile([NOBS, 128], f32, tag="pp")
                    nc.tensor.transpose(pyt[:], yt[:], ident[:])
                    nc.vector.tensor_copy(
                        inpT[0:64, b * Tt + tch * 128: b * Tt + (tch + 1) * 128],
                        pyt[:],
                    )

            # ---- phase 3: GRU ----
            # PSUM layout per block (two banks, double-buffered):
            #   rz bank [128, 512]: cols = (tile mi 0..7, step k 0..7, batch b 0..7)
            #   nx bank [128, 512]: cols 0:256   = hn (tile i, k, b)
            #                       cols 256:512 = xn (tile i, k, b)
            # Bank init: one K=8 matmul  bias8.T @ indicator  (start=True zeroes
            # the bank), then x-proj GEMM and recurrent matmuls accumulate.
            R = C * 8  # 64 cols per (tile, block)

            def hslot(t):
                return outsT[:, t * 32:(t + 1) * 32]

            def kslice(ap_full, base, k, ntile):
                # cols {base + i*R + k*8 + b} as (tile outer, batch inner)
                return bass.AP(
                    tensor=ap_full.tensor,
                    offset=ap_full.offset + base + k * 8,
                    ap=[list(ap_full.ap[0]), [R, ntile], [1, 8]],
                )

            def ap2d(t, ncol_outer, stride_outer):
                # view of a flat [128, ncol_outer*8] work tile as 2D free dims
                return bass.AP(
                    tensor=t.tensor, offset=t.offset,
                    ap=[list(t.ap[0]), [stride_outer, ncol_outer], [1, 8]],
                )

            def xp_block(jb, rzp, nxp):
                t0 = jb * C
                ia = inpT[:]
                rhs_inp = bass.AP(
                    tensor=ia.tensor, offset=ia.offset + t0,
                    ap=[list(ia.ap[0]), [1, C], [Tt, BS]],
                )
                # bank-wide bias init (one matmul per bank)
                nc.tensor.matmul(rzp[:], brz8[:], ind[:], start=True, stop=False)
                nc.tensor.matmul(nxp[:], bnx8[:], ind[:], start=True, stop=False)
                # x-projection GEMM (accumulates)
                for m in range(8):
                    nc.tensor.matmul(
                        rzp[:, m * R:(m + 1) * R],
                        wih[:, m * 128:(m + 1) * 128],
                        rhs_inp, start=False, stop=False,
                    )
                for m in range(8, 12):
                    nc.tensor.matmul(
                        nxp[:, 256 + (m - 8) * R:256 + (m - 7) * R],
                        wih[:, m * 128:(m + 1) * 128],
                        rhs_inp, start=False, stop=False,
                    )

            def fc_block(jb):
                # outs rows for steps t0..t0+7 live in hslot(t0+1..t0+8)
                t0 = jb * C
                oa = outsT[:]
                psf = pfc.tile([MST, 64], f32, tag="fc")
                for kc in range(4):
                    rhs = bass.AP(
                        tensor=oa.tensor,
                        offset=oa.offset + (t0 + 1) * 32 + kc * 8,
                        ap=[list(oa.ap[0]), [1, BS], [32, C]],
                    )  # (b outer, t inner)
                    nc.tensor.matmul(
                        psf[:], fcw[:, kc * MST:(kc + 1) * MST], rhs,
                        start=(kc == 0), stop=(kc == 3),
                    )
                oT = work.tile([MST, 64], f32, tag="oT")
                xa = XpT[:]
                xp_ap = bass.AP(
                    tensor=xa.tensor, offset=xa.offset + t0,
                    ap=[list(xa.ap[0]), [Tt, BS], [1, C]],
                )
                nc.vector.scalar_tensor_tensor(
                    ap2d(oT, BS, 8), psf[:], fcb[:], xp_ap,
                    op0=mybir.AluOpType.add, op1=mybir.AluOpType.add,
                )
                ptr = pfc.tile([MST, 64], f32, tag="tr")
                nc.tensor.transpose(ptr[:], oT[:], ident[0:64, 0:64])
                ot = work.tile([MST, 64], f32, tag="ot")
                nc.vector.tensor_copy(ot[:], ptr[:])
                for b in range(BS):
                    nc.sync.dma_start(
                        out_d[b, t0:t0 + C, :], ot[b * 8:(b + 1) * 8, :]
                    )

            Sig = mybir.ActivationFunctionType.Sigmoid
            Tanh = mybir.ActivationFunctionType.Tanh
            Copy = mybir.ActivationFunctionType.Copy

            rz_cur = pgates.tile([128, 512], f32, tag="rz")
            nx_cur = pgates.tile([128, 512], f32, tag="nx")
            xp_block(0, rz_cur, nx_cur)

            for j in range(nblk):
                rzp, nxp = rz_cur, nx_cur
                for k in range(C):
                    t = j * C + k
                    h_rd = hslot(t)
                    last_k = (k == C - 1)

                    # r,z tiles first: sigmoid overlaps the hn matmuls
                    for mi in range(8):
                        for kc in range(4):
                            nc.tensor.matmul(
                                rzp[:, mi * R + k * 8:mi * R + k * 8 + 8],
                                whh[:, (kc * 12 + mi) * 128:(kc * 12 + mi + 1) * 128],
                                h_rd[:, kc * 8:(kc + 1) * 8],
                                start=False,
                                stop=(last_k and mi == 7 and kc == 3),
                            )
                    for i in range(4):
                        for kc in range(4):
                            nc.tensor.matmul(
                                nxp[:, i * R + k * 8:i * R + k * 8 + 8],
                                whh[:, (kc * 12 + 8 + i) * 128:(kc * 12 + 9 + i) * 128],
                                h_rd[:, kc * 8:(kc + 1) * 8],
                                start=False,
                                stop=(last_k and i == 3 and kc == 3),
                            )

                    # interleaved fillers for the tensor-idle tail windows
                    if k == 1 and j + 1 < nblk:
                        rz_cur = pgates.tile([128, 512], f32, tag="rz")
                        nx_cur = pgates.tile([128, 512], f32, tag="nx")
                        xp_block(j + 1, rz_cur, nx_cur)
                    if k == 3 and j >= 1:
                        fc_block(j - 1)

                    # ---- per-step chain ----
                    rza = work.tile([128, 64], f32, tag="rza")
                    nc.scalar.activation(ap2d(rza, 8, 8), kslice(rzp[:], 0, k, 8), Sig)
                    t1 = work.tile([128, 32], f32, tag="t1")
                    nc.vector.tensor_mul(
                        ap2d(t1, 4, 8), ap2d(rza, 4, 8), kslice(nxp[:], 0, k, 4)
                    )
                    t2 = work.tile([128, 32], f32, tag="t2")
                    nc.vector.tensor_add(
                        ap2d(t2, 4, 8), ap2d(t1, 4, 8), kslice(nxp[:], 256, k, 4)
                    )
                    # 1 - z == sigmoid(-(xz+hz)): second ACT on the PSUM z-slice
                    omz = work.tile([128, 32], f32, tag="omz")
                    nc.scalar.activation(
                        ap2d(omz, 4, 8), kslice(rzp[:], 4 * R, k, 4), Sig, scale=-1.0
                    )
                    n_t = work.tile([128, 32], f32, tag="n")
                    nc.scalar.activation(n_t[:], t2[:], Tanh)
                    zh = work.tile([128, 32], f32, tag="zh")
                    nc.vector.tensor_mul(zh[:], rza[:, 32:64], h_rd)
                    u_t = work.tile([128, 32], f32, tag="u")
                    nc.vector.tensor_mul(u_t[:], n_t[:], omz[:])
                    nc.vector.tensor_add(hslot(t + 1), u_t[:], zh[:])

            fc_block(nblk - 1)

    nc.compile()
    return nc


def _prep_weights(F_mat, W_ih, W_hh, b_ih, b_hh, fc_W, fc_b):
    bf = ml_dtypes.bfloat16
    FkT = np.empty((MST, 65 * MST), np.float32)
    P = np.eye(MST, dtype=np.float32)
    for k in range(65):
        FkT[:, k * MST:(k + 1) * MST] = P.T
        P = (F_mat @ P).astype(np.float32)
    WihT = np.ascontiguousarray(W_ih.T).astype(bf)
    WhhT = np.empty((128, 48 * 128), bf)
    for kc in range(4):
        for m in range(12):
            blk = W_hh[m * 128:(m + 1) * 128, kc * 128:(kc + 1) * 128]
            WhhT[:, (kc * 12 + m) * 128:(kc * 12 + m) * 128 + 128] = blk.T.astype(bf)
    # rz bank bias rows: tile mi -> (b_ih + b_hh)[mi*128:(mi+1)*128]
    brz8 = (b_ih + b_hh)[:2 * HID].reshape(8, 128).astype(bf)
    # nx bank bias rows: 0..3 = b_hh n-tiles (hn), 4..7 = b_ih n-tiles (xn)
    bnx8 = np.concatenate(
        [b_hh[2 * HID:].reshape(4, 128), b_ih[2 * HID:].reshape(4, 128)], axis=0
    ).astype(bf)
    ind = np.zeros((8, 512), np.float32)
    for jj in range(8):
        ind[jj, jj * 64:(jj + 1) * 64] = 1.0
    ind = ind.astype(bf)
    fcWT = np.empty((128, 4 * MST), bf)
    for kc in range(4):
        fcWT[:, kc * MST:(kc + 1) * MST] = fc_W[:, kc * 128:(kc + 1) * 128].T.astype(bf)
    fcb = fc_b.reshape(MST, 1).astype(np.float32)
    return dict(FkT=FkT, WihT=WihT, WhhT=WhhT, brz8=brz8, bnx8=bnx8, ind=ind,
                fcWT=fcWT, fcb=fcb)


def kernel(Y, x0_hat, F_mat, W_ih, W_hh, b_ih, b_hh, fc_W, fc_b):
    from concourse.bass_utils import run_bass_kernel_spmd

    t_steps = Y.shape[1]
    if t_steps not in _compiled:
        _compiled[t_steps] = _build_bass(t_steps)
    nc = _compiled[t_steps]

    w = _prep_weights(F_mat, W_ih, W_hh, b_ih, b_hh, fc_W, fc_b)
    in_maps = []
    for c in range(NCORES):
        sl = slice(c * BS, (c + 1) * BS)
        in_maps.append({
            "Y": np.ascontiguousarray(Y[sl]).astype(np.float32),
            "x0T": np.ascontiguousarray(x0_hat[sl].T).astype(np.float32),
            **w,
        })
    trace = os.environ.get("KTRACE") == "1"
    res = run_bass_kernel_spmd(nc, in_maps, list(range(NCORES)), trace=trace)
    global LAST_RESULTS
    LAST_RESULTS = res
    out = np.concatenate([res.results[c]["out"] for c in range(NCORES)], axis=0)
    return out.astype(np.float32)


if __name__ == "__main__":
    rng = np.random.default_rng(0)
    ins = {
        "Y": rng.standard_normal((B, int(os.environ.get("KT", T)), NOBS), dtype=np.float32),
        "x0_hat": rng.standard_normal((B, MST), dtype=np.float32),
        "F_mat": (0.99 * np.linalg.qr(rng.standard_normal((MST, MST)))[0]).astype(np.float32),
        "W_ih": 0.05 * rng.standard_normal((H3, 128), dtype=np.float32),
        "W_hh": 0.05 * rng.standard_normal((H3, HID), dtype=np.float32),
        "b_ih": 0.05 * rng.standard_normal(H3, dtype=np.float32),
        "b_hh": 0.05 * rng.standard_normal(H3, dtype=np.float32),
        "fc_W": 0.05 * rng.standard_normal((MST, HID), dtype=np.float32),
        "fc_b": 0.05 * rng.standard_normal(MST, dtype=np.float32),
    }
    print(kernel(**ins).shape)
